# revision 38
# baseline (speedup 1.0000x reference)
"""Trainium2 Bass kernel for nn_Decoder (recursive tree GRU decoder).

Self-contained: builds + compiles + runs a Bass/Tile kernel SPMD on 8
NeuronCores, pure data-parallel over the batch dim.

Math (per batch element, mirroring the reference):
  hidden0 = z @ z2h_w
  preorder tree of depth DEPTH / arity ARITY; at each node v:
    pred_v = h_v @ h2o_w                     (output logits)
    probs_v = softmax(pred_v)
    child0 = GRU_anc(probs_v, h_v)
    hf = child0_h; for sibling c = 1..arity-1:
      hf = GRU_frat(probs_{child c-1}, hf)
      child_c = tanh(hf @ uf_w + h_v @ ua_w)

Schedule: BFS level-order.  The recursion's only cross-subtree dependency is
each node's own softmax (not its subtree's), so all nodes of a level run as
parallel waves instead of a serial tree walk.

Layout: feature-major [feat(128 partitions), batch(free)], batch tile BT=512,
8 trees per core, two tree "slots" interleaved at emission time.  Everything
bf16 except PSUM (fp32, TRN2 requirement) and the softmax reciprocal.
Per-level hiddens live in [128,1024] bf16 pair tiles (2 nodes/tile); GRU
pointwise ops batch both nodes of a wave into single 1024-col instructions.

Matmul packing: GRU input matmuls contract K=32 (probs) -> row-tiled
(tile_position=(32*band,0)); pred matmuls have M=32 -> col-tiled, 4 nodes'
preds pack into one PSUM bank.  No PE transposes anywhere: z is transposed
on host, preds are written packed and reordered on host.

Engines: ACT does all tanh/exp, DVE does PSUM-side ALU + copies + reciprocal,
GpSimd(Pool) does SBUF-side bf16 ALU.
"""

import os

# Reset NeuronCores at device open: protects against a wedged device state
# inherited from a previous process (observed intermittently on this host).
os.environ.setdefault("NEURON_RT_RESET_CORES", "1")

from collections import deque
from contextlib import ExitStack

import numpy as np

import concourse.bass as bass
import concourse.mybir as mybir
from concourse import tile
from concourse.bass_utils import run_bass_kernel_spmd

F32 = mybir.dt.float32
BF16 = mybir.dt.bfloat16
AF = mybir.ActivationFunctionType
ALU = mybir.AluOpType

B, I, H, O = 32768, 128, 128, 32
N_CORES = 8
B_CORE = B // N_CORES          # 4096
BT = 512
TREES = B_CORE // BT           # 8
SLOTS = 2
NA_IN_PSUM = True
RZ_WIDE = False    # one [128,2048] rz tile (1 TRZ instr) vs two [128,1024]
RZ_BUFS = 2
CD_BUFS = 2
PU_TAG = "cd"      # tag for stage-P / U / h0 psum tiles
PU_BUFS = None     # None -> follow CD_BUFS when PU_TAG=="cd"
PS_ONE = False     # single [128,2048] psum tag for everything
Y_GW = False       # yield inside gru_wave after TRZ
Y_P1 = False       # yield in stage_p after pred mm
Y_P2 = False       # yield in stage_p before S mm
Y_U = False        # yield in u_stage between mms and tanh
PR_ENGINE = "vector"  # engine for probs-mult: "gpsimd" | "vector"
RC_BF16 = True     # reciprocal output dtype bf16
SP_BUFS = 2
TRZ_BUFS = 2
NN_BUFS = 3
P_LAG = 1
U_LAG = 1
EX_BUFS = 2
DZ_POOL = False  # d op engine: False=DVE, True=gpsimd, "alt"=alternate
ZT_POOL = True   # zt input ready early (after TRZ); Pool latency hides behind m/nn chain
ZD_POOL = False
HP_POOL = False
EXP_SKIP = True   # leaf-last-block packs: DVE pred copy instead of ACT exp
Z_PREFETCH = False # one wide z DMA per iteration instead of per-tree loads (sim-neutral, unverified on HW)
OUT_DMA_ENG = "sync"  # queue for out DMAs: "sync" | "gpsimd"
_dz_ctr = [0]

_PE_OPS = ("InstMatmult", "InstLdweights", "InstMatmultMx")

# weight column layout in wA [128, WCOLS]
_WC_Z2H = 0
_WC_S = 128
_WC_H2O = 256
_WC_UF = 288
_WC_UA = 416
_WC_GRU = 544                  # per gru g: wiRep 3*128 | wh 3*128
_GRU_STRIDE = 768
WCOLS = _WC_GRU + 2 * _GRU_STRIDE  # 2080


def _split_multi_waits(nc):
    """This container's walrus accepts at most 1 embedded sem wait on most
    instructions (0 on self-loading matmuls) and <=2 on a standalone
    EventSemaphore.  Tile emits multi-waits; split them."""
    for f in nc.m.functions:
        for bb in f.blocks:
            insts = bb.instructions
            new = []
            changed = False
            for ins in insts:
                si = ins.sync_info
                ow = list(si.on_wait) if si is not None and si.on_wait else []
                movable = [w for w in ow if w.wait_reg is None]
                fixed = [w for w in ow if w.wait_reg is not None]
                opc = type(ins).__name__
                limit = 0 if opc in _PE_OPS else 1
                limit = max(0, limit - len(fixed))
                if len(movable) > limit:
                    keep = movable[:limit]
                    move = movable[limit:]
                    for i in range(0, len(move), 2):
                        ev = mybir.InstEventSemaphore(
                            name=f"{ins.name}-wsp{i}",
                            ins=[],
                            outs=[],
                            sync_info=mybir.SyncInfo(
                                on_wait=move[i : i + 2], on_update=[]
                            ),
                        )
                        ev.engine = ins.engine
                        new.append(ev)
                    upd = list(si.on_update) if si.on_update else []
                    ins.sync_info = mybir.SyncInfo(on_wait=fixed + keep, on_update=upd)
                    changed = True
                new.append(ins)
            if changed:
                bb.instructions = new


def _preorder_map(depth, arity):
    """(level, pos) -> preorder index; pos of child c of parent p at level L
    is c*arity**L + p."""
    pre = {}
    ctr = [0]

    def rec(L, pos, d):
        pre[(L, pos)] = ctr[0]
        ctr[0] += 1
        if d == 0:
            return
        base = arity**L
        for c in range(arity):
            rec(L + 1, c * base + pos, d - 1)

    rec(0, 0, depth)
    return pre, ctr[0]


def _plan_packs(depth, arity):
    """Static plan of output packs: list of (level, [positions]), <=4 nodes,
    never spanning sibling blocks."""
    packs = [(0, [0])]
    for L in range(1, depth + 1):
        base = arity ** (L - 1)
        for c in range(arity):
            for s in range(0, base, 4):
                packs.append((L, [c * base + i for i in range(s, min(s + 4, base))]))
    return packs


def build(depth, arity, loop_n=1):
    """Build the per-core Bass module.  Returns (nc, n_nodes, out_map):
    out_map = list of (pack_slot, band, level, pos)."""
    pre_map, nn_ = _preorder_map(depth, arity)
    packs = _plan_packs(depth, arity)
    n_packs = len(packs)
    pack_slot = {}
    out_map = []
    for si, (L, poss) in enumerate(packs):
        pack_slot[(L, poss[0])] = si
        for band, p in enumerate(poss):
            out_map.append((si, band, L, p))

    raw_slots = set()

    nc = bass.Bass(trn_type="TRN2")

    zT_d = nc.dram_tensor("zT", [128, B_CORE], BF16, kind="ExternalInput")
    wA_d = nc.dram_tensor("wA", [128, WCOLS], BF16, kind="ExternalInput")
    out_d = nc.dram_tensor("out", [n_packs, TREES, 128, BT], BF16, kind="ExternalOutput")

    with tile.TileContext(nc) as tc:
        with (
            tc.tile_pool(name="wp", bufs=1) as wp,
            tc.tile_pool(name="hp", bufs=1) as hp,
            tc.tile_pool(name="prp", bufs=1) as prp,
            tc.tile_pool(name="sp", bufs=SP_BUFS) as sp,
            tc.tile_pool(name="pp", bufs=2, space="PSUM") as pp,
        ):
            wA = wp.tile([128, WCOLS], BF16, tag="wA")
            nc.sync.dma_start(wA[:], wA_d[:])

            w_z2h = wA[:, _WC_Z2H : _WC_Z2H + 128]
            w_S = wA[:, _WC_S : _WC_S + 128]
            w_h2o = wA[:, _WC_H2O : _WC_H2O + 32]
            w_uf = wA[:, _WC_UF : _WC_UF + 128]
            w_ua = wA[:, _WC_UA : _WC_UA + 128]

            def w_gi(g, k, band):
                c0 = _WC_GRU + g * _GRU_STRIDE + k * 128
                return wA[32 * band : 32 * band + 32, c0 : c0 + 128]

            def w_gh(g, k):
                c0 = _WC_GRU + g * _GRU_STRIDE + 384 + k * 128
                return wA[:, c0 : c0 + 128]

            _ls = ExitStack()
            if loop_n > 1:
                _ls.enter_context(tc.For_i(0, loop_n, 1))

            zT_sb = None
            if Z_PREFETCH:
                zT_sb = sp.tile([128, B_CORE], BF16, tag="zT_sb", name="zT_sb", bufs=1)
                nc.sync.dma_start(zT_sb[:], zT_d[:])

            def tree_gen(slot, tree):
                Hs = {L: {} for L in range(depth + 1)}   # L -> {q: pair tile}
                PR = {L: {} for L in range(depth + 1)}   # L -> {pos: (tile, band)}

                def h_tile(L, q):
                    d = Hs[L]
                    if q not in d:
                        # leaf pairs wholly inside the last sibling block die
                        # right after their pred pack -> transient tags
                        transient = (
                            L == depth
                            and depth >= 2
                            and 2 * q >= (arity - 1) * arity ** (depth - 1)
                        )
                        if transient:
                            tg = f"hL{slot}_{q % 4}"
                        else:
                            tg = f"h{slot}_{L % 2}_{q}"
                        d[q] = hp.tile([128, 1024], BF16, tag=tg, name=tg)
                    return d[q]

                def h_ref(L, pos):
                    q, r = divmod(pos, 2)
                    return h_tile(L, q)[:, r * 512 : (r + 1) * 512]

                def stage_p(L, poss, need_probs):
                    """pred + (optional) softmax for <=4 nodes of one block.
                    Generator: yields between pipeline phases."""
                    t = (pp.tile([128, 2048], F32, tag="ps", name="tps", bufs=2)[:, 0:1024] if PS_ONE else pp.tile([128, 1024], F32, tag=PU_TAG, name="tps", bufs=(PU_BUFS if PU_BUFS is not None else (RZ_BUFS if PU_TAG == "rz" else CD_BUFS))))
                    for j, p in enumerate(poss):
                        nc.tensor.matmul(
                            t[32 * j : 32 * j + 32, 0:BT],
                            w_h2o,
                            h_ref(L, p),
                            start=True,
                            stop=True,
                            tile_position=(0, 32 * j),
                            skip_group_check=True,
                        )
                    if Y_P1:
                        yield
                    if EXP_SKIP and not need_probs:
                        raw_slots.add(pack_slot[(L, poss[0])])
                        po = sp.tile([128, BT], BF16, tag=f"ex{slot}", name=f"po{slot}", bufs=EX_BUFS)
                        nc.vector.tensor_copy(out=po[:], in_=t[:, 0:BT])
                        (nc.gpsimd if OUT_DMA_ENG == "gpsimd" else nc.sync).dma_start(out_d[pack_slot[(L, poss[0])], tree], po[:])
                        return
                    ex = sp.tile([128, BT], BF16, tag=f"ex{slot}", name=f"ex{slot}", bufs=EX_BUFS)
                    nc.scalar.activation(ex[:], t[:, 0:BT], AF.Exp, bias=0.0, scale=1.0)
                    (nc.gpsimd if OUT_DMA_ENG == "gpsimd" else nc.sync).dma_start(out_d[pack_slot[(L, poss[0])], tree], ex[:])
                    if not need_probs:
                        return
                    if Y_P2:
                        yield
                    nc.tensor.matmul(t[:, BT:1024], w_S, ex[:], start=True, stop=True)
                    rc = sp.tile([128, BT], BF16 if RC_BF16 else F32,
                                 tag=f"rc{slot}", name=f"rc{slot}", bufs=EX_BUFS)
                    if RC_BF16:
                        with nc.allow_low_precision(reason="softmax denom, rel tol 2e-2"):
                            nc.vector.reciprocal(rc[:], t[:, BT:1024])
                    else:
                        nc.vector.reciprocal(rc[:], t[:, BT:1024])
                    pr = prp.tile([128, BT], BF16, tag=f"pr{slot}_{L % 2}_{poss[0]}", name=f"pr{slot}")
                    eng = nc.gpsimd if PR_ENGINE == "gpsimd" else nc.vector
                    eng.tensor_tensor(out=pr[:], in0=ex[:], in1=rc[:], op=ALU.mult)
                    for j, p in enumerate(poss):
                        PR[L][p] = (pr, j)

                def gru_wave(g, prd, pnodes, h_in, h_out):
                    """One GRU wave over 1-2 nodes.
                    prd: probs dict, pnodes: positions keying prd.
                    h_in/h_out: contiguous APs [128, 512*w]."""
                    w = len(pnodes)
                    cw = w * BT
                    pb = [prd[pos] for pos in pnodes]
                    if PS_ONE:
                        t_ps = pp.tile([128, 2048], F32, tag="ps", name="t_ps", bufs=2)
                        rseg = lambda k, j: t_ps[:, k * 1024 + j * BT : k * 1024 + (j + 1) * BT]
                    elif RZ_WIDE:
                        t_rz = pp.tile([128, 2048], F32, tag="rz", name="t_rz",
                                       bufs=RZ_BUFS)
                        rseg = lambda k, j: t_rz[:, k * 1024 + j * BT : k * 1024 + (j + 1) * BT]
                    else:
                        t_r = pp.tile([128, 1024], F32, tag="rz", name="t_r",
                                      bufs=RZ_BUFS)
                        t_z = pp.tile([128, 1024], F32, tag="rz", name="t_z",
                                      bufs=RZ_BUFS)
                        rseg = lambda k, j: (t_r if k == 0 else t_z)[:, j * BT : (j + 1) * BT]
                    # k-outer emission: row-tile pairs issue concurrently,
                    # full-array matmuls share LDWEIGHTS
                    for k in (0, 1):
                        for j in range(w):
                            prt, band = pb[j]
                            nc.tensor.matmul(
                                rseg(k, j),
                                w_gi(g, k, band),
                                prt[32 * band : 32 * band + 32, :],
                                start=True, stop=False,
                                tile_position=(32 * band, 0),
                                skip_group_check=True)
                    for k in (0, 1):
                        for j in range(w):
                            nc.tensor.matmul(
                                rseg(k, j),
                                w_gh(g, k), h_in[:, j * BT : (j + 1) * BT],
                                start=False, stop=True,
                                skip_group_check=True)
                    trz = sp.tile([128, 2048], BF16, tag=f"trz{slot}", name="trz", bufs=TRZ_BUFS)
                    if PS_ONE and w == 2:
                        nc.scalar.activation(trz[:], t_ps[:], AF.Tanh,
                                             bias=0.0, scale=0.5)
                    elif PS_ONE:
                        nc.scalar.activation(trz[:, 0:cw], t_ps[:, 0:cw], AF.Tanh,
                                             bias=0.0, scale=0.5)
                        nc.scalar.activation(trz[:, 1024 : 1024 + cw],
                                             t_ps[:, 1024 : 1024 + cw],
                                             AF.Tanh, bias=0.0, scale=0.5)
                    elif RZ_WIDE and w == 2:
                        nc.scalar.activation(trz[:], t_rz[:], AF.Tanh,
                                             bias=0.0, scale=0.5)
                    else:
                        nc.scalar.activation(trz[:, 0:cw], rseg(0, 0).outer(w) if False else (t_rz[:, 0:cw] if RZ_WIDE else t_r[:, 0:cw]), AF.Tanh,
                                             bias=0.0, scale=0.5)
                        nc.scalar.activation(trz[:, 1024 : 1024 + cw],
                                             (t_rz[:, 1024 : 1024 + cw] if RZ_WIDE else t_z[:, 0:cw]),
                                             AF.Tanh, bias=0.0, scale=0.5)
                    if Y_GW:
                        yield
                    # cd tile: gh2 matmuls set has_written, DVE rewrites it
                    # in place as (t_r+1)*gh2 = r*gh2 (bits stay set), then
                    # the gi2 row-tiles accumulate on top -> na in PSUM.
                    if PS_ONE:
                        t_cd = t_ps[:, 0:1024]
                    else:
                        t_cd = pp.tile([128, 1024], F32, tag="cd", name="t_cd", bufs=CD_BUFS)
                    for j in range(w):
                        nc.tensor.matmul(t_cd[:, j * BT : (j + 1) * BT],
                                         w_gh(g, 2), h_in[:, j * BT : (j + 1) * BT],
                                         start=True, stop=(not NA_IN_PSUM),
                                         skip_group_check=True)
                    if NA_IN_PSUM:
                        nc.vector.scalar_tensor_tensor(
                            out=t_cd[:, 0:cw], in0=trz[:, 0:cw], scalar=1.0,
                            in1=t_cd[:, 0:cw], op0=ALU.add, op1=ALU.mult)
                        for j in range(w):
                            prt, band = pb[j]
                            nc.tensor.matmul(t_cd[:, j * BT : (j + 1) * BT],
                                             w_gi(g, 2, band),
                                             prt[32 * band : 32 * band + 32, :],
                                             start=False, stop=True,
                                             tile_position=(32 * band, 0),
                                             skip_group_check=True)
                        na_src = t_cd
                        na_space_psum = True
                    else:
                        t_gi = pp.tile([128, 1024], F32, tag="cd", name="t_gi", bufs=CD_BUFS)
                        for j in range(w):
                            prt, band = pb[j]
                            nc.tensor.matmul(t_gi[:, j * BT : (j + 1) * BT],
                                             w_gi(g, 2, band),
                                             prt[32 * band : 32 * band + 32, :],
                                             start=True, stop=True,
                                             tile_position=(32 * band, 0),
                                             skip_group_check=True)
                        m_t = sp.tile([128, 1024], BF16, tag=f"m{slot}", name="m_t")
                        nc.vector.scalar_tensor_tensor(
                            out=m_t[:, 0:cw], in0=trz[:, 0:cw], scalar=1.0,
                            in1=t_cd[:, 0:cw], op0=ALU.add, op1=ALU.mult)
                        na_t = sp.tile([128, 1024], BF16, tag=f"na{slot}", name="na_t")
                        nc.vector.tensor_tensor(out=na_t[:, 0:cw], in0=m_t[:, 0:cw],
                                                in1=t_gi[:, 0:cw], op=ALU.add)
                        na_src = na_t
                    nn_t = sp.tile([128, 1024], BF16, tag=f"nn{slot}", name="nn_t", bufs=NN_BUFS)
                    nc.scalar.activation(nn_t[:, 0:cw], na_src[:, 0:cw], AF.Tanh,
                                         bias=0.0, scale=1.0)
                    d_t = sp.tile([128, 1024], BF16, tag=f"d{slot}", name="d_t")
                    if DZ_POOL == "alt":
                        _dz_ctr[0] += 1
                        _deng = nc.gpsimd if _dz_ctr[0] % 2 else nc.vector
                    else:
                        _deng = nc.gpsimd if DZ_POOL else nc.vector
                    _deng.tensor_tensor(out=d_t[:, 0:cw], in0=h_in[:, 0:cw],
                                            in1=nn_t[:, 0:cw], op=ALU.subtract)
                    z_t = sp.tile([128, 1024], BF16, tag=f"zt{slot}", name="z_t")
                    (nc.gpsimd if ZT_POOL else nc.vector).tensor_scalar(
                        out=z_t[:, 0:cw],
                        in0=trz[:, 1024 : 1024 + cw],
                        scalar1=0.5, scalar2=0.5,
                        op0=ALU.mult, op1=ALU.add)
                    zd_t = sp.tile([128, 1024], BF16, tag=f"zd{slot}", name="zd_t")
                    (nc.gpsimd if ZD_POOL else nc.vector).tensor_tensor(
                        out=zd_t[:, 0:cw], in0=z_t[:, 0:cw],
                        in1=d_t[:, 0:cw], op=ALU.mult)
                    (nc.gpsimd if HP_POOL else nc.vector).tensor_tensor(
                        out=h_out[:, 0:cw], in0=zd_t[:, 0:cw],
                        in1=nn_t[:, 0:cw], op=ALU.add)

                # ---- hidden0 ----
                if Z_PREFETCH:
                    zt = zT_sb[:, tree * BT : (tree + 1) * BT]
                else:
                    zt = sp.tile([128, BT], BF16, tag=f"z{slot}", name=f"zt{slot}")
                    nc.sync.dma_start(zt[:], zT_d[:, tree * BT : (tree + 1) * BT])
                t0 = (pp.tile([128, 2048], F32, tag="ps", name="t0", bufs=2)[:, 0:1024] if PS_ONE else pp.tile([128, 1024], F32, tag=PU_TAG, name="t0", bufs=(PU_BUFS if PU_BUFS is not None else (RZ_BUFS if PU_TAG == "rz" else CD_BUFS))))
                nc.tensor.matmul(t0[:, 0:BT], w_z2h, zt[:], start=True, stop=True)
                nc.vector.tensor_copy(out=h_ref(0, 0), in_=t0[:, 0:BT])
                yield
                yield from stage_p(0, [0], depth > 0)
                yield

                for L in range(depth):
                    npar = arity**L
                    child_L = L + 1
                    leafc = child_L == depth
                    # ---- STAGE A + P(block 0); P lags one pack behind ----
                    need0 = (not leafc) or arity > 1
                    pq = deque()
                    for s in range(0, npar, 4):
                        hi = min(s + 4, npar)
                        for q0 in range(s, hi, 2):
                            w = min(2, npar - q0)
                            pn = list(range(q0, q0 + w))
                            yield from gru_wave(0, PR[L], pn,
                                     h_tile(L, q0 // 2)[:, 0 : 512 * w],
                                     h_tile(child_L, q0 // 2)[:, 0 : 512 * w])
                            yield
                            if len(pq) >= P_LAG:
                                yield from stage_p(*pq.popleft())
                                yield
                        pq.append((child_L, list(range(s, hi)), need0))
                    while pq:
                        yield from stage_p(*pq.popleft())
                        yield
                    # ---- sibling chain: F-wave, U lags 1 wave, P lags 1 pack ----
                    hf_prev = None
                    for c in range(1, arity):
                        needc = (not leafc) or (c < arity - 1)
                        hf_tiles = {}
                        uq = deque()
                        pq = deque()

                        def u_stage(hf_t, q, q0, w, c=c, npar=npar,
                                    child_L=child_L, L=L):
                            tu = (pp.tile([128, 2048], F32, tag="ps", name="tu", bufs=2)[:, 0:1024] if PS_ONE else pp.tile([128, 1024], F32, tag=PU_TAG, name="tu", bufs=(PU_BUFS if PU_BUFS is not None else (RZ_BUFS if PU_TAG == "rz" else CD_BUFS))))
                            for j in range(w):
                                seg = slice(j * BT, (j + 1) * BT)
                                nc.tensor.matmul(tu[:, seg], w_uf, hf_t[:, seg],
                                                 start=True, stop=False)
                                nc.tensor.matmul(tu[:, seg], w_ua,
                                                 h_tile(L, q)[:, seg],
                                                 start=False, stop=True)
                            if Y_U:
                                yield
                            cpos = c * npar + q0
                            for j in range(w):
                                qg, rg = divmod(cpos + j, 2)
                                if j == 0 and rg == 0 and w == 2:
                                    nc.scalar.activation(
                                        h_tile(child_L, qg)[:, 0:1024],
                                        tu[:, 0:1024], AF.Tanh,
                                        bias=0.0, scale=1.0)
                                    break
                                nc.scalar.activation(
                                    h_ref(child_L, cpos + j),
                                    tu[:, j * BT : (j + 1) * BT], AF.Tanh,
                                    bias=0.0, scale=1.0)

                        for s in range(0, npar, 4):
                            hi = min(s + 4, npar)
                            for q0 in range(s, hi, 2):
                                q = q0 // 2
                                w = min(2, npar - q0)
                                pn = [(c - 1) * npar + i for i in range(q0, q0 + w)]
                                if c == 1:
                                    hf_in = h_tile(child_L, q)[:, 0 : 512 * w]
                                else:
                                    hf_in = hf_prev[q][:, 0 : 512 * w]
                                hf_t = hp.tile([128, 1024], BF16,
                                               tag=f"hf{slot}_{c % 2}_{q % 4}",
                                               name="hf_t")
                                hf_tiles[q] = hf_t
                                yield from gru_wave(1, PR[child_L], pn, hf_in,
                                         hf_t[:, 0 : 512 * w])
                                yield
                                if len(uq) >= U_LAG:
                                    yield from u_stage(*uq.popleft())
                                    yield
                                uq.append((hf_t, q, q0, w))
                            if len(pq) >= P_LAG:
                                yield from stage_p(*pq.popleft())
                                yield
                            pq.append((child_L,
                                       [c * npar + i for i in range(s, hi)],
                                       needc))
                        while uq:
                            yield from u_stage(*uq.popleft())
                            yield
                        while pq:
                            yield from stage_p(*pq.popleft())
                            yield
                        hf_prev = hf_tiles

            # driver: interleave SLOTS tree generators
            pending = deque(range(TREES))
            slots_free = list(range(SLOTS))
            active = []
            while pending or active:
                while pending and slots_free:
                    s = slots_free.pop()
                    active.append((s, tree_gen(s, pending.popleft())))
                for item in list(active):
                    s, g = item
                    try:
                        next(g)
                    except StopIteration:
                        active.remove(item)
                        slots_free.append(s)

            _ls.close()

    _split_multi_waits(nc)
    return nc, nn_, out_map, raw_slots


def _prep_weights(inputs):
    """Host-side weight packing into wA [128, WCOLS] bf16."""
    import ml_dtypes

    f = lambda x: np.asarray(x, dtype=np.float32)
    wA = np.zeros((128, WCOLS), np.float32)
    wA[:, _WC_Z2H : _WC_Z2H + 128] = f(inputs["z2h_w"])
    S = np.zeros((128, 128), np.float32)
    for t in range(4):
        S[t * O : (t + 1) * O, t * O : (t + 1) * O] = 1.0
    wA[:, _WC_S : _WC_S + 128] = S
    wA[:, _WC_H2O : _WC_H2O + 32] = f(inputs["h2o_w"])
    wA[:, _WC_UF : _WC_UF + 128] = f(inputs["uf_w"])
    wA[:, _WC_UA : _WC_UA + 128] = f(inputs["ua_w"])
    for g, name in enumerate(("anc", "frat")):
        wi = f(inputs[f"{name}_wi"])  # [3, O, H]
        wh = f(inputs[f"{name}_wh"])  # [3, H, H]
        base = _WC_GRU + g * _GRU_STRIDE
        for k in range(3):
            for band in range(4):
                wA[32 * band : 32 * band + 32,
                   base + k * 128 : base + (k + 1) * 128] = wi[k]
            scale = 0.5 if k == 2 else 1.0
            wA[:, base + 384 + k * 128 : base + 384 + (k + 1) * 128] = scale * wh[k]
    return {"wA": wA.astype(ml_dtypes.bfloat16)}


def make_in_maps(inputs):
    import ml_dtypes

    w = _prep_weights(inputs)
    z = np.asarray(inputs["z"], dtype=np.float32).reshape(B, I)
    in_maps = []
    for c in range(N_CORES):
        im = dict(w)
        zc = z[c * B_CORE : (c + 1) * B_CORE]  # [B_CORE, I]
        im["zT"] = np.ascontiguousarray(zc.T.astype(ml_dtypes.bfloat16))
        in_maps.append(im)
    return in_maps


def unpack_out(res_list, out_map, depth, arity, raw_slots=()):
    """res_list: per-core arrays [n_packs, TREES, 128, BT] (bf16); slots not
    in raw_slots hold exp(pred) and get log-recovered.
    Returns [n_nodes, B, 1, O] f32 in preorder."""
    pre_map, nn_ = _preorder_map(depth, arity)
    out = np.empty((nn_, B, 1, O), np.float32)
    for c, arr in enumerate(res_list):
        a = np.asarray(arr).astype(np.float32)
        for si in range(a.shape[0]):
            if si not in raw_slots:
                a[si] = np.log(a[si])
        for (si, band, L, p) in out_map:
            pre = pre_map[(L, p)]
            seg = a[si, :, band * 32 : (band + 1) * 32, :]  # [TREES, 32, BT]
            out[pre, c * B_CORE : (c + 1) * B_CORE, 0, :] = (
                seg.transpose(0, 2, 1).reshape(B_CORE, O)
            )
    return out


_BUILD_CACHE = {}


def _get_built(depth, arity):
    key = (depth, arity)
    if key not in _BUILD_CACHE:
        _BUILD_CACHE[key] = build(depth, arity)
    return _BUILD_CACHE[key]


def kernel(**inputs) -> np.ndarray:
    depth = int(np.asarray(inputs["depth"]))
    arity = int(np.asarray(inputs["arity"]))
    for bname in ("z2h_b", "h2o_b", "anc_bi", "anc_bh",
                  "frat_bi", "frat_bh", "ua_b", "uf_b"):
        if bname in inputs and np.any(np.asarray(inputs[bname])):
            raise NotImplementedError(f"nonzero bias {bname} not supported")

    nc, nn_, out_map, raw_slots = _get_built(depth, arity)
    in_maps = make_in_maps(inputs)
    last_err = None
    for attempt in range(3):
        try:
            res = run_bass_kernel_spmd(nc, in_maps, core_ids=list(range(N_CORES)))
            res_list = [res.results[c]["out"] for c in range(N_CORES)]
            return unpack_out(res_list, out_map, depth, arity, raw_slots)
        except Exception as e:  # transient NRT/device errors: retry
            last_err = e
            import time as _time
            _time.sleep(5)
    raise last_err


if __name__ == "__main__":
    rng = np.random.default_rng(0)
    ins = {
        "z": rng.standard_normal((B, 1, I)).astype(np.float32),
        "z2h_w": rng.standard_normal((I, H)).astype(np.float32) * 0.08,
        "z2h_b": np.zeros(H, np.float32),
        "h2o_w": rng.standard_normal((H, O)).astype(np.float32) * 0.1,
        "h2o_b": np.zeros(O, np.float32),
        "anc_wi": rng.standard_normal((3, O, H)).astype(np.float32) * 0.1,
        "anc_wh": rng.standard_normal((3, H, H)).astype(np.float32) * 0.08,
        "anc_bi": np.zeros((3, H), np.float32),
        "anc_bh": np.zeros((3, H), np.float32),
        "frat_wi": rng.standard_normal((3, O, H)).astype(np.float32) * 0.1,
        "frat_wh": rng.standard_normal((3, H, H)).astype(np.float32) * 0.08,
        "frat_bi": np.zeros((3, H), np.float32),
        "frat_bh": np.zeros((3, H), np.float32),
        "ua_w": rng.standard_normal((H, H)).astype(np.float32) * 0.08,
        "ua_b": np.zeros(H, np.float32),
        "uf_w": rng.standard_normal((H, H)).astype(np.float32) * 0.08,
        "uf_b": np.zeros(H, np.float32),
        "depth": np.int64(2),
        "arity": np.int64(2),
    }
    out = kernel(**ins)
    print("out shape:", out.shape, "finite:", np.isfinite(out).all())


# revision 39
# speedup vs baseline: 10.0921x; 10.0921x over previous
"""Trainium2 Bass kernel for nn_Decoder (recursive tree GRU decoder).

Self-contained: builds + compiles + runs a Bass/Tile kernel SPMD on 8
NeuronCores, pure data-parallel over the batch dim.

Math (per batch element, mirroring the reference):
  hidden0 = z @ z2h_w
  preorder tree of depth DEPTH / arity ARITY; at each node v:
    pred_v = h_v @ h2o_w                     (output logits)
    probs_v = softmax(pred_v)
    child0 = GRU_anc(probs_v, h_v)
    hf = child0_h; for sibling c = 1..arity-1:
      hf = GRU_frat(probs_{child c-1}, hf)
      child_c = tanh(hf @ uf_w + h_v @ ua_w)

Schedule: BFS level-order.  The recursion's only cross-subtree dependency is
each node's own softmax (not its subtree's), so all nodes of a level run as
parallel waves instead of a serial tree walk.

Layout: feature-major [feat(128 partitions), batch(free)], batch tile BT=512,
8 trees per core, two tree "slots" interleaved at emission time.  Everything
bf16 except PSUM (fp32, TRN2 requirement) and the softmax reciprocal.
Per-level hiddens live in [128,1024] bf16 pair tiles (2 nodes/tile); GRU
pointwise ops batch both nodes of a wave into single 1024-col instructions.

Matmul packing: GRU input matmuls contract K=32 (probs) -> row-tiled
(tile_position=(32*band,0)); pred matmuls have M=32 -> col-tiled, 4 nodes'
preds pack into one PSUM bank.  No PE transposes anywhere: z is transposed
on host, preds are written packed and reordered on host.

Engines: ACT does all tanh/exp, DVE does PSUM-side ALU + copies + reciprocal,
GpSimd(Pool) does SBUF-side bf16 ALU.
"""

import os

# Reset NeuronCores at device open: protects against a wedged device state
# inherited from a previous process (observed intermittently on this host).
os.environ.setdefault("NEURON_RT_RESET_CORES", "1")

from collections import deque
from contextlib import ExitStack

import numpy as np

import concourse.bass as bass
import concourse.mybir as mybir
from concourse import tile
from concourse.bass_utils import run_bass_kernel_spmd

F32 = mybir.dt.float32
BF16 = mybir.dt.bfloat16
AF = mybir.ActivationFunctionType
ALU = mybir.AluOpType

B, I, H, O = 32768, 128, 128, 32
N_CORES = 8
B_CORE = B // N_CORES          # 4096
BT = 512
TREES = B_CORE // BT           # 8
SLOTS = 2
NA_IN_PSUM = True
RZ_WIDE = False    # one [128,2048] rz tile (1 TRZ instr) vs two [128,1024]
RZ_BUFS = 2
CD_BUFS = 2
PU_TAG = "cd"      # tag for stage-P / U / h0 psum tiles
PU_BUFS = None     # None -> follow CD_BUFS when PU_TAG=="cd"
PS_ONE = False     # single [128,2048] psum tag for everything
Y_GW = False       # yield inside gru_wave after TRZ
Y_P1 = False       # yield in stage_p after pred mm
Y_P2 = False       # yield in stage_p before S mm
Y_U = False        # yield in u_stage between mms and tanh
PR_ENGINE = "vector"  # engine for probs-mult: "gpsimd" | "vector"
RC_BF16 = True     # reciprocal output dtype bf16
SP_BUFS = 2
TRZ_BUFS = 2
NN_BUFS = 3
P_LAG = 1
U_LAG = 1
EX_BUFS = 2
DZ_POOL = False  # d op engine: False=DVE, True=gpsimd, "alt"=alternate
ZT_POOL = True   # zt input ready early (after TRZ); Pool latency hides behind m/nn chain
ZD_POOL = False
HP_POOL = False
EXP_SKIP = True   # leaf-last-block packs: DVE pred copy instead of ACT exp
Z_PREFETCH = False # one wide z DMA per iteration instead of per-tree loads (sim-neutral, unverified on HW)
OUT_DMA_ENG = "sync"  # queue for out DMAs: "sync" | "gpsimd"
DRV_STEPS = 1      # generator yield-steps per slot per driver round
_dz_ctr = [0]

_PE_OPS = ("InstMatmult", "InstLdweights", "InstMatmultMx")

# weight column layout in wA [128, WCOLS]
_WC_Z2H = 0
_WC_S = 128
_WC_H2O = 256
_WC_UF = 288
_WC_UA = 416
_WC_GRU = 544                  # per gru g: wiRep 3*128 | wh 3*128
_GRU_STRIDE = 768
WCOLS = _WC_GRU + 2 * _GRU_STRIDE  # 2080


def _split_multi_waits(nc):
    """This container's walrus accepts at most 1 embedded sem wait on most
    instructions (0 on self-loading matmuls) and <=2 on a standalone
    EventSemaphore.  Tile emits multi-waits; split them."""
    for f in nc.m.functions:
        for bb in f.blocks:
            insts = bb.instructions
            new = []
            changed = False
            for ins in insts:
                si = ins.sync_info
                ow = list(si.on_wait) if si is not None and si.on_wait else []
                movable = [w for w in ow if w.wait_reg is None]
                fixed = [w for w in ow if w.wait_reg is not None]
                opc = type(ins).__name__
                limit = 0 if opc in _PE_OPS else 1
                limit = max(0, limit - len(fixed))
                if len(movable) > limit:
                    keep = movable[:limit]
                    move = movable[limit:]
                    for i in range(0, len(move), 2):
                        ev = mybir.InstEventSemaphore(
                            name=f"{ins.name}-wsp{i}",
                            ins=[],
                            outs=[],
                            sync_info=mybir.SyncInfo(
                                on_wait=move[i : i + 2], on_update=[]
                            ),
                        )
                        ev.engine = ins.engine
                        new.append(ev)
                    upd = list(si.on_update) if si.on_update else []
                    ins.sync_info = mybir.SyncInfo(on_wait=fixed + keep, on_update=upd)
                    changed = True
                new.append(ins)
            if changed:
                bb.instructions = new


def _preorder_map(depth, arity):
    """(level, pos) -> preorder index; pos of child c of parent p at level L
    is c*arity**L + p."""
    pre = {}
    ctr = [0]

    def rec(L, pos, d):
        pre[(L, pos)] = ctr[0]
        ctr[0] += 1
        if d == 0:
            return
        base = arity**L
        for c in range(arity):
            rec(L + 1, c * base + pos, d - 1)

    rec(0, 0, depth)
    return pre, ctr[0]


def _plan_packs(depth, arity):
    """Static plan of output packs: list of (level, [positions]), <=4 nodes,
    never spanning sibling blocks."""
    packs = [(0, [0])]
    for L in range(1, depth + 1):
        base = arity ** (L - 1)
        for c in range(arity):
            for s in range(0, base, 4):
                packs.append((L, [c * base + i for i in range(s, min(s + 4, base))]))
    return packs


def build(depth, arity, loop_n=1):
    """Build the per-core Bass module.  Returns (nc, n_nodes, out_map):
    out_map = list of (pack_slot, band, level, pos)."""
    pre_map, nn_ = _preorder_map(depth, arity)
    packs = _plan_packs(depth, arity)
    n_packs = len(packs)
    pack_slot = {}
    out_map = []
    for si, (L, poss) in enumerate(packs):
        pack_slot[(L, poss[0])] = si
        for band, p in enumerate(poss):
            out_map.append((si, band, L, p))

    raw_slots = set()

    nc = bass.Bass(trn_type="TRN2")

    zT_d = nc.dram_tensor("zT", [128, B_CORE], BF16, kind="ExternalInput")
    wA_d = nc.dram_tensor("wA", [128, WCOLS], BF16, kind="ExternalInput")
    out_d = nc.dram_tensor("out", [n_packs, TREES, 128, BT], BF16, kind="ExternalOutput")

    with tile.TileContext(nc) as tc:
        with (
            tc.tile_pool(name="wp", bufs=1) as wp,
            tc.tile_pool(name="hp", bufs=1) as hp,
            tc.tile_pool(name="prp", bufs=1) as prp,
            tc.tile_pool(name="sp", bufs=SP_BUFS) as sp,
            tc.tile_pool(name="pp", bufs=2, space="PSUM") as pp,
        ):
            wA = wp.tile([128, WCOLS], BF16, tag="wA")
            nc.sync.dma_start(wA[:], wA_d[:])

            w_z2h = wA[:, _WC_Z2H : _WC_Z2H + 128]
            w_S = wA[:, _WC_S : _WC_S + 128]
            w_h2o = wA[:, _WC_H2O : _WC_H2O + 32]
            w_uf = wA[:, _WC_UF : _WC_UF + 128]
            w_ua = wA[:, _WC_UA : _WC_UA + 128]

            def w_gi(g, k, band):
                c0 = _WC_GRU + g * _GRU_STRIDE + k * 128
                return wA[32 * band : 32 * band + 32, c0 : c0 + 128]

            def w_gh(g, k):
                c0 = _WC_GRU + g * _GRU_STRIDE + 384 + k * 128
                return wA[:, c0 : c0 + 128]

            _ls = ExitStack()
            if loop_n > 1:
                _ls.enter_context(tc.For_i(0, loop_n, 1))

            zT_sb = None
            if Z_PREFETCH:
                zT_sb = sp.tile([128, B_CORE], BF16, tag="zT_sb", name="zT_sb", bufs=1)
                nc.sync.dma_start(zT_sb[:], zT_d[:])

            def tree_gen(slot, tree):
                Hs = {L: {} for L in range(depth + 1)}   # L -> {q: pair tile}
                PR = {L: {} for L in range(depth + 1)}   # L -> {pos: (tile, band)}

                def h_tile(L, q):
                    d = Hs[L]
                    if q not in d:
                        # leaf pairs wholly inside the last sibling block die
                        # right after their pred pack -> transient tags
                        transient = (
                            L == depth
                            and depth >= 2
                            and 2 * q >= (arity - 1) * arity ** (depth - 1)
                        )
                        if transient:
                            tg = f"hL{slot}_{q % 4}"
                        else:
                            tg = f"h{slot}_{L % 2}_{q}"
                        d[q] = hp.tile([128, 1024], BF16, tag=tg, name=tg)
                    return d[q]

                def h_ref(L, pos):
                    q, r = divmod(pos, 2)
                    return h_tile(L, q)[:, r * 512 : (r + 1) * 512]

                def stage_p(L, poss, need_probs):
                    """pred + (optional) softmax for <=4 nodes of one block.
                    Generator: yields between pipeline phases."""
                    t = (pp.tile([128, 2048], F32, tag="ps", name="tps", bufs=2)[:, 0:1024] if PS_ONE else pp.tile([128, 1024], F32, tag=PU_TAG, name="tps", bufs=(PU_BUFS if PU_BUFS is not None else (RZ_BUFS if PU_TAG == "rz" else CD_BUFS))))
                    for j, p in enumerate(poss):
                        nc.tensor.matmul(
                            t[32 * j : 32 * j + 32, 0:BT],
                            w_h2o,
                            h_ref(L, p),
                            start=True,
                            stop=True,
                            tile_position=(0, 32 * j),
                            skip_group_check=True,
                        )
                    if Y_P1:
                        yield
                    if EXP_SKIP and not need_probs:
                        raw_slots.add(pack_slot[(L, poss[0])])
                        po = sp.tile([128, BT], BF16, tag=f"ex{slot}", name=f"po{slot}", bufs=EX_BUFS)
                        nc.vector.tensor_copy(out=po[:], in_=t[:, 0:BT])
                        (nc.gpsimd if OUT_DMA_ENG == "gpsimd" else nc.sync).dma_start(out_d[pack_slot[(L, poss[0])], tree], po[:])
                        return
                    ex = sp.tile([128, BT], BF16, tag=f"ex{slot}", name=f"ex{slot}", bufs=EX_BUFS)
                    nc.scalar.activation(ex[:], t[:, 0:BT], AF.Exp, bias=0.0, scale=1.0)
                    (nc.gpsimd if OUT_DMA_ENG == "gpsimd" else nc.sync).dma_start(out_d[pack_slot[(L, poss[0])], tree], ex[:])
                    if not need_probs:
                        return
                    if Y_P2:
                        yield
                    nc.tensor.matmul(t[:, BT:1024], w_S, ex[:], start=True, stop=True)
                    rc = sp.tile([128, BT], BF16 if RC_BF16 else F32,
                                 tag=f"rc{slot}", name=f"rc{slot}", bufs=EX_BUFS)
                    if RC_BF16:
                        with nc.allow_low_precision(reason="softmax denom, rel tol 2e-2"):
                            nc.vector.reciprocal(rc[:], t[:, BT:1024])
                    else:
                        nc.vector.reciprocal(rc[:], t[:, BT:1024])
                    pr = prp.tile([128, BT], BF16, tag=f"pr{slot}_{L % 2}_{poss[0]}", name=f"pr{slot}")
                    eng = nc.gpsimd if PR_ENGINE == "gpsimd" else nc.vector
                    eng.tensor_tensor(out=pr[:], in0=ex[:], in1=rc[:], op=ALU.mult)
                    for j, p in enumerate(poss):
                        PR[L][p] = (pr, j)

                def gru_wave(g, prd, pnodes, h_in, h_out):
                    """One GRU wave over 1-2 nodes.
                    prd: probs dict, pnodes: positions keying prd.
                    h_in/h_out: contiguous APs [128, 512*w]."""
                    w = len(pnodes)
                    cw = w * BT
                    pb = [prd[pos] for pos in pnodes]
                    if PS_ONE:
                        t_ps = pp.tile([128, 2048], F32, tag="ps", name="t_ps", bufs=2)
                        rseg = lambda k, j: t_ps[:, k * 1024 + j * BT : k * 1024 + (j + 1) * BT]
                    elif RZ_WIDE:
                        t_rz = pp.tile([128, 2048], F32, tag="rz", name="t_rz",
                                       bufs=RZ_BUFS)
                        rseg = lambda k, j: t_rz[:, k * 1024 + j * BT : k * 1024 + (j + 1) * BT]
                    else:
                        t_r = pp.tile([128, 1024], F32, tag="rz", name="t_r",
                                      bufs=RZ_BUFS)
                        t_z = pp.tile([128, 1024], F32, tag="rz", name="t_z",
                                      bufs=RZ_BUFS)
                        rseg = lambda k, j: (t_r if k == 0 else t_z)[:, j * BT : (j + 1) * BT]
                    # k-outer emission: row-tile pairs issue concurrently,
                    # full-array matmuls share LDWEIGHTS
                    for k in (0, 1):
                        for j in range(w):
                            prt, band = pb[j]
                            nc.tensor.matmul(
                                rseg(k, j),
                                w_gi(g, k, band),
                                prt[32 * band : 32 * band + 32, :],
                                start=True, stop=False,
                                tile_position=(32 * band, 0),
                                skip_group_check=True)
                    for k in (0, 1):
                        for j in range(w):
                            nc.tensor.matmul(
                                rseg(k, j),
                                w_gh(g, k), h_in[:, j * BT : (j + 1) * BT],
                                start=False, stop=True,
                                skip_group_check=True)
                    trz = sp.tile([128, 2048], BF16, tag=f"trz{slot}", name="trz", bufs=TRZ_BUFS)
                    if PS_ONE and w == 2:
                        nc.scalar.activation(trz[:], t_ps[:], AF.Tanh,
                                             bias=0.0, scale=0.5)
                    elif PS_ONE:
                        nc.scalar.activation(trz[:, 0:cw], t_ps[:, 0:cw], AF.Tanh,
                                             bias=0.0, scale=0.5)
                        nc.scalar.activation(trz[:, 1024 : 1024 + cw],
                                             t_ps[:, 1024 : 1024 + cw],
                                             AF.Tanh, bias=0.0, scale=0.5)
                    elif RZ_WIDE and w == 2:
                        nc.scalar.activation(trz[:], t_rz[:], AF.Tanh,
                                             bias=0.0, scale=0.5)
                    else:
                        nc.scalar.activation(trz[:, 0:cw], rseg(0, 0).outer(w) if False else (t_rz[:, 0:cw] if RZ_WIDE else t_r[:, 0:cw]), AF.Tanh,
                                             bias=0.0, scale=0.5)
                        nc.scalar.activation(trz[:, 1024 : 1024 + cw],
                                             (t_rz[:, 1024 : 1024 + cw] if RZ_WIDE else t_z[:, 0:cw]),
                                             AF.Tanh, bias=0.0, scale=0.5)
                    if Y_GW:
                        yield
                    # cd tile: gh2 matmuls set has_written, DVE rewrites it
                    # in place as (t_r+1)*gh2 = r*gh2 (bits stay set), then
                    # the gi2 row-tiles accumulate on top -> na in PSUM.
                    if PS_ONE:
                        t_cd = t_ps[:, 0:1024]
                    else:
                        t_cd = pp.tile([128, 1024], F32, tag="cd", name="t_cd", bufs=CD_BUFS)
                    for j in range(w):
                        nc.tensor.matmul(t_cd[:, j * BT : (j + 1) * BT],
                                         w_gh(g, 2), h_in[:, j * BT : (j + 1) * BT],
                                         start=True, stop=(not NA_IN_PSUM),
                                         skip_group_check=True)
                    if NA_IN_PSUM:
                        nc.vector.scalar_tensor_tensor(
                            out=t_cd[:, 0:cw], in0=trz[:, 0:cw], scalar=1.0,
                            in1=t_cd[:, 0:cw], op0=ALU.add, op1=ALU.mult)
                        for j in range(w):
                            prt, band = pb[j]
                            nc.tensor.matmul(t_cd[:, j * BT : (j + 1) * BT],
                                             w_gi(g, 2, band),
                                             prt[32 * band : 32 * band + 32, :],
                                             start=False, stop=True,
                                             tile_position=(32 * band, 0),
                                             skip_group_check=True)
                        na_src = t_cd
                        na_space_psum = True
                    else:
                        t_gi = pp.tile([128, 1024], F32, tag="cd", name="t_gi", bufs=CD_BUFS)
                        for j in range(w):
                            prt, band = pb[j]
                            nc.tensor.matmul(t_gi[:, j * BT : (j + 1) * BT],
                                             w_gi(g, 2, band),
                                             prt[32 * band : 32 * band + 32, :],
                                             start=True, stop=True,
                                             tile_position=(32 * band, 0),
                                             skip_group_check=True)
                        m_t = sp.tile([128, 1024], BF16, tag=f"m{slot}", name="m_t")
                        nc.vector.scalar_tensor_tensor(
                            out=m_t[:, 0:cw], in0=trz[:, 0:cw], scalar=1.0,
                            in1=t_cd[:, 0:cw], op0=ALU.add, op1=ALU.mult)
                        na_t = sp.tile([128, 1024], BF16, tag=f"na{slot}", name="na_t")
                        nc.vector.tensor_tensor(out=na_t[:, 0:cw], in0=m_t[:, 0:cw],
                                                in1=t_gi[:, 0:cw], op=ALU.add)
                        na_src = na_t
                    nn_t = sp.tile([128, 1024], BF16, tag=f"nn{slot}", name="nn_t", bufs=NN_BUFS)
                    nc.scalar.activation(nn_t[:, 0:cw], na_src[:, 0:cw], AF.Tanh,
                                         bias=0.0, scale=1.0)
                    d_t = sp.tile([128, 1024], BF16, tag=f"d{slot}", name="d_t")
                    if DZ_POOL == "alt":
                        _dz_ctr[0] += 1
                        _deng = nc.gpsimd if _dz_ctr[0] % 2 else nc.vector
                    else:
                        _deng = nc.gpsimd if DZ_POOL else nc.vector
                    _deng.tensor_tensor(out=d_t[:, 0:cw], in0=h_in[:, 0:cw],
                                            in1=nn_t[:, 0:cw], op=ALU.subtract)
                    z_t = sp.tile([128, 1024], BF16, tag=f"zt{slot}", name="z_t")
                    (nc.gpsimd if ZT_POOL else nc.vector).tensor_scalar(
                        out=z_t[:, 0:cw],
                        in0=trz[:, 1024 : 1024 + cw],
                        scalar1=0.5, scalar2=0.5,
                        op0=ALU.mult, op1=ALU.add)
                    zd_t = sp.tile([128, 1024], BF16, tag=f"zd{slot}", name="zd_t")
                    (nc.gpsimd if ZD_POOL else nc.vector).tensor_tensor(
                        out=zd_t[:, 0:cw], in0=z_t[:, 0:cw],
                        in1=d_t[:, 0:cw], op=ALU.mult)
                    (nc.gpsimd if HP_POOL else nc.vector).tensor_tensor(
                        out=h_out[:, 0:cw], in0=zd_t[:, 0:cw],
                        in1=nn_t[:, 0:cw], op=ALU.add)

                # ---- hidden0 ----
                if Z_PREFETCH:
                    zt = zT_sb[:, tree * BT : (tree + 1) * BT]
                else:
                    zt = sp.tile([128, BT], BF16, tag=f"z{slot}", name=f"zt{slot}")
                    nc.sync.dma_start(zt[:], zT_d[:, tree * BT : (tree + 1) * BT])
                t0 = (pp.tile([128, 2048], F32, tag="ps", name="t0", bufs=2)[:, 0:1024] if PS_ONE else pp.tile([128, 1024], F32, tag=PU_TAG, name="t0", bufs=(PU_BUFS if PU_BUFS is not None else (RZ_BUFS if PU_TAG == "rz" else CD_BUFS))))
                nc.tensor.matmul(t0[:, 0:BT], w_z2h, zt[:], start=True, stop=True)
                nc.vector.tensor_copy(out=h_ref(0, 0), in_=t0[:, 0:BT])
                yield
                yield from stage_p(0, [0], depth > 0)
                yield

                for L in range(depth):
                    npar = arity**L
                    child_L = L + 1
                    leafc = child_L == depth
                    # ---- STAGE A + P(block 0); P lags one pack behind ----
                    need0 = (not leafc) or arity > 1
                    pq = deque()
                    for s in range(0, npar, 4):
                        hi = min(s + 4, npar)
                        for q0 in range(s, hi, 2):
                            w = min(2, npar - q0)
                            pn = list(range(q0, q0 + w))
                            yield from gru_wave(0, PR[L], pn,
                                     h_tile(L, q0 // 2)[:, 0 : 512 * w],
                                     h_tile(child_L, q0 // 2)[:, 0 : 512 * w])
                            yield
                            if len(pq) >= P_LAG:
                                yield from stage_p(*pq.popleft())
                                yield
                        pq.append((child_L, list(range(s, hi)), need0))
                    while pq:
                        yield from stage_p(*pq.popleft())
                        yield
                    # ---- sibling chain: F-wave, U lags 1 wave, P lags 1 pack ----
                    hf_prev = None
                    for c in range(1, arity):
                        needc = (not leafc) or (c < arity - 1)
                        hf_tiles = {}
                        uq = deque()
                        pq = deque()

                        def u_stage(hf_t, q, q0, w, c=c, npar=npar,
                                    child_L=child_L, L=L):
                            tu = (pp.tile([128, 2048], F32, tag="ps", name="tu", bufs=2)[:, 0:1024] if PS_ONE else pp.tile([128, 1024], F32, tag=PU_TAG, name="tu", bufs=(PU_BUFS if PU_BUFS is not None else (RZ_BUFS if PU_TAG == "rz" else CD_BUFS))))
                            for j in range(w):
                                seg = slice(j * BT, (j + 1) * BT)
                                nc.tensor.matmul(tu[:, seg], w_uf, hf_t[:, seg],
                                                 start=True, stop=False)
                                nc.tensor.matmul(tu[:, seg], w_ua,
                                                 h_tile(L, q)[:, seg],
                                                 start=False, stop=True)
                            if Y_U:
                                yield
                            cpos = c * npar + q0
                            for j in range(w):
                                qg, rg = divmod(cpos + j, 2)
                                if j == 0 and rg == 0 and w == 2:
                                    nc.scalar.activation(
                                        h_tile(child_L, qg)[:, 0:1024],
                                        tu[:, 0:1024], AF.Tanh,
                                        bias=0.0, scale=1.0)
                                    break
                                nc.scalar.activation(
                                    h_ref(child_L, cpos + j),
                                    tu[:, j * BT : (j + 1) * BT], AF.Tanh,
                                    bias=0.0, scale=1.0)

                        for s in range(0, npar, 4):
                            hi = min(s + 4, npar)
                            for q0 in range(s, hi, 2):
                                q = q0 // 2
                                w = min(2, npar - q0)
                                pn = [(c - 1) * npar + i for i in range(q0, q0 + w)]
                                if c == 1:
                                    hf_in = h_tile(child_L, q)[:, 0 : 512 * w]
                                else:
                                    hf_in = hf_prev[q][:, 0 : 512 * w]
                                hf_t = hp.tile([128, 1024], BF16,
                                               tag=f"hf{slot}_{c % 2}_{q % 4}",
                                               name="hf_t")
                                hf_tiles[q] = hf_t
                                yield from gru_wave(1, PR[child_L], pn, hf_in,
                                         hf_t[:, 0 : 512 * w])
                                yield
                                if len(uq) >= U_LAG:
                                    yield from u_stage(*uq.popleft())
                                    yield
                                uq.append((hf_t, q, q0, w))
                            if len(pq) >= P_LAG:
                                yield from stage_p(*pq.popleft())
                                yield
                            pq.append((child_L,
                                       [c * npar + i for i in range(s, hi)],
                                       needc))
                        while uq:
                            yield from u_stage(*uq.popleft())
                            yield
                        while pq:
                            yield from stage_p(*pq.popleft())
                            yield
                        hf_prev = hf_tiles

            # driver: interleave SLOTS tree generators
            pending = deque(range(TREES))
            slots_free = list(range(SLOTS))
            active = []
            while pending or active:
                while pending and slots_free:
                    s = slots_free.pop()
                    active.append((s, tree_gen(s, pending.popleft())))
                for item in list(active):
                    s, g = item
                    try:
                        for _ in range(DRV_STEPS):
                            next(g)
                    except StopIteration:
                        active.remove(item)
                        slots_free.append(s)

            _ls.close()

    _split_multi_waits(nc)
    return nc, nn_, out_map, raw_slots


def _prep_weights(inputs):
    """Host-side weight packing into wA [128, WCOLS] bf16."""
    import ml_dtypes

    f = lambda x: np.asarray(x, dtype=np.float32)
    wA = np.zeros((128, WCOLS), np.float32)
    wA[:, _WC_Z2H : _WC_Z2H + 128] = f(inputs["z2h_w"])
    S = np.zeros((128, 128), np.float32)
    for t in range(4):
        S[t * O : (t + 1) * O, t * O : (t + 1) * O] = 1.0
    wA[:, _WC_S : _WC_S + 128] = S
    wA[:, _WC_H2O : _WC_H2O + 32] = f(inputs["h2o_w"])
    wA[:, _WC_UF : _WC_UF + 128] = f(inputs["uf_w"])
    wA[:, _WC_UA : _WC_UA + 128] = f(inputs["ua_w"])
    for g, name in enumerate(("anc", "frat")):
        wi = f(inputs[f"{name}_wi"])  # [3, O, H]
        wh = f(inputs[f"{name}_wh"])  # [3, H, H]
        base = _WC_GRU + g * _GRU_STRIDE
        for k in range(3):
            for band in range(4):
                wA[32 * band : 32 * band + 32,
                   base + k * 128 : base + (k + 1) * 128] = wi[k]
            scale = 0.5 if k == 2 else 1.0
            wA[:, base + 384 + k * 128 : base + 384 + (k + 1) * 128] = scale * wh[k]
    return {"wA": wA.astype(ml_dtypes.bfloat16)}


def make_in_maps(inputs):
    import ml_dtypes

    w = _prep_weights(inputs)
    z = np.asarray(inputs["z"], dtype=np.float32).reshape(B, I)
    in_maps = []
    for c in range(N_CORES):
        im = dict(w)
        zc = z[c * B_CORE : (c + 1) * B_CORE]  # [B_CORE, I]
        im["zT"] = np.ascontiguousarray(zc.T.astype(ml_dtypes.bfloat16))
        in_maps.append(im)
    return in_maps


def unpack_out(res_list, out_map, depth, arity, raw_slots=()):
    """res_list: per-core arrays [n_packs, TREES, 128, BT] (bf16); slots not
    in raw_slots hold exp(pred) and get log-recovered.
    Returns [n_nodes, B, 1, O] f32 in preorder."""
    pre_map, nn_ = _preorder_map(depth, arity)
    out = np.empty((nn_, B, 1, O), np.float32)
    for c, arr in enumerate(res_list):
        a = np.asarray(arr).astype(np.float32)
        for si in range(a.shape[0]):
            if si not in raw_slots:
                a[si] = np.log(a[si])
        for (si, band, L, p) in out_map:
            pre = pre_map[(L, p)]
            seg = a[si, :, band * 32 : (band + 1) * 32, :]  # [TREES, 32, BT]
            out[pre, c * B_CORE : (c + 1) * B_CORE, 0, :] = (
                seg.transpose(0, 2, 1).reshape(B_CORE, O)
            )
    return out


_BUILD_CACHE = {}


def _get_built(depth, arity):
    key = (depth, arity)
    if key not in _BUILD_CACHE:
        _BUILD_CACHE[key] = build(depth, arity)
    return _BUILD_CACHE[key]


def kernel(**inputs) -> np.ndarray:
    depth = int(np.asarray(inputs["depth"]))
    arity = int(np.asarray(inputs["arity"]))
    for bname in ("z2h_b", "h2o_b", "anc_bi", "anc_bh",
                  "frat_bi", "frat_bh", "ua_b", "uf_b"):
        if bname in inputs and np.any(np.asarray(inputs[bname])):
            raise NotImplementedError(f"nonzero bias {bname} not supported")

    nc, nn_, out_map, raw_slots = _get_built(depth, arity)
    in_maps = make_in_maps(inputs)
    last_err = None
    for attempt in range(3):
        try:
            res = run_bass_kernel_spmd(nc, in_maps, core_ids=list(range(N_CORES)))
            res_list = [res.results[c]["out"] for c in range(N_CORES)]
            return unpack_out(res_list, out_map, depth, arity, raw_slots)
        except Exception as e:  # transient NRT/device errors: retry
            last_err = e
            import time as _time
            _time.sleep(5)
    raise last_err


if __name__ == "__main__":
    rng = np.random.default_rng(0)
    ins = {
        "z": rng.standard_normal((B, 1, I)).astype(np.float32),
        "z2h_w": rng.standard_normal((I, H)).astype(np.float32) * 0.08,
        "z2h_b": np.zeros(H, np.float32),
        "h2o_w": rng.standard_normal((H, O)).astype(np.float32) * 0.1,
        "h2o_b": np.zeros(O, np.float32),
        "anc_wi": rng.standard_normal((3, O, H)).astype(np.float32) * 0.1,
        "anc_wh": rng.standard_normal((3, H, H)).astype(np.float32) * 0.08,
        "anc_bi": np.zeros((3, H), np.float32),
        "anc_bh": np.zeros((3, H), np.float32),
        "frat_wi": rng.standard_normal((3, O, H)).astype(np.float32) * 0.1,
        "frat_wh": rng.standard_normal((3, H, H)).astype(np.float32) * 0.08,
        "frat_bi": np.zeros((3, H), np.float32),
        "frat_bh": np.zeros((3, H), np.float32),
        "ua_w": rng.standard_normal((H, H)).astype(np.float32) * 0.08,
        "ua_b": np.zeros(H, np.float32),
        "uf_w": rng.standard_normal((H, H)).astype(np.float32) * 0.08,
        "uf_b": np.zeros(H, np.float32),
        "depth": np.int64(2),
        "arity": np.int64(2),
    }
    out = kernel(**ins)
    print("out shape:", out.shape, "finite:", np.isfinite(out).all())
